# revision 2
# baseline (speedup 1.0000x reference)
"""Trainium2 Bass kernel for a 4-layer dense transformer LM (BitWhisker).

Strategy: sequence-parallel over 8 cores (2 batches x 4 chunks of 256 tokens).
Per layer: replicated weights (bf16), feature-major activations [D, T] so
RMSNorm / rope / attention need no on-chip transposes. K/V exchanged between
the 4 cores of each batch with one AllGather per layer. Final (tied) vocab
projection is computed per-core for its own 256 tokens (no communication).

kernel(**inputs) takes the FULL fp32 inputs and returns full [B,S,V] logits.
"""

import os
import numpy as np
import ml_dtypes

import concourse.bass as bass
import concourse.tile as tile
import concourse.mybir as mybir
from concourse import bacc, bass_utils

BF16 = ml_dtypes.bfloat16
F32 = mybir.dt.float32
BF = mybir.dt.bfloat16

V = 32000
B = 2
S = 1024
D = 1024
H = 16
HD = 64
L = 4
FF = 2816
THETA = 10000.0
EPS = 1e-6

P = 128
T = 256            # local tokens per core
KC = D // P        # 8 chunks of D
FC = FF // P       # 22 chunks of FF
NCORES = 8
NCHUNK = 4         # sequence chunks per batch
RG = [[0, 1, 2, 3], [4, 5, 6, 7]]
NEG = -1.0e30

_CACHE = {}


def _build(l_use=L, v_use=V):
    """Build + compile the Bass program (same program for all 8 cores)."""
    nc = bacc.Bacc("TRN2", target_bir_lowering=False, debug=False,
                   enable_asserts=False, num_devices=NCORES)

    def din(name, shape, dt):
        return nc.dram_tensor(name, shape, dt, kind="ExternalInput").ap()

    h0T = din("h0T", [D, T], F32)
    nvt = max(1, (v_use + NCORES * 512 - 1) // (NCORES * 512))  # vocab tiles per core
    wq_i = din("wq", [l_use, P, KC, D], BF)
    wk_i = din("wk", [l_use, P, KC, D], BF)
    wv_i = din("wv", [l_use, P, KC, D], BF)
    wo_i = din("wo", [l_use, P, KC, D], BF)
    w1_i = din("w1c", [l_use, FC, P, KC, P], BF)
    w3_i = din("w3c", [l_use, FC, P, KC, P], BF)
    w2_i = din("w2c", [l_use, KC, P, FC, P], BF)
    emb_i = din("embT", [P, nvt, KC, 512], BF)
    cd_i = din("cdup", [P, T], F32)
    sd_i = din("sdup", [P, T], F32)
    pm_i = din("perm", [P, P], BF)
    tri_i = din("tri", [2, P, T], BF)
    bv_i = din("biasv", [P, NCHUNK], F32)
    out_e = nc.dram_tensor("logits_loc", [B * S, nvt * 512], mybir.dt.float16,
                           kind="ExternalOutput").ap()

    from contextlib import ExitStack
    with tile.TileContext(nc) as tc, ExitStack() as ctx:
        cpool = ctx.enter_context(tc.tile_pool(name="consts", bufs=1))
        hpool = ctx.enter_context(tc.tile_pool(name="hres", bufs=1))
        apool = ctx.enter_context(tc.tile_pool(name="acts", bufs=1))
        wpool = ctx.enter_context(tc.tile_pool(name="w4", bufs=2))
        w13p = ctx.enter_context(tc.tile_pool(name="w13", bufs=4))
        w2p = ctx.enter_context(tc.tile_pool(name="w2", bufs=3))
        embp = ctx.enter_context(tc.tile_pool(name="embp", bufs=2))
        tmp = ctx.enter_context(tc.tile_pool(name="tmp", bufs=2))
        etmp = ctx.enter_context(tc.tile_pool(name="etmp", bufs=3))
        opool = ctx.enter_context(tc.tile_pool(name="outp", bufs=3))
        kvp = ctx.enter_context(tc.tile_pool(name="kvglob", bufs=1))
        dram = ctx.enter_context(tc.tile_pool(name="dram", bufs=3, space="DRAM"))
        psA = ctx.enter_context(tc.tile_pool(name="psA", bufs=3, space="PSUM"))
        psAV = ctx.enter_context(tc.tile_pool(name="psAV", bufs=2, space="PSUM"))
        psB = psA

        # ---- constants ----
        cd = cpool.tile([P, T], F32); nc.sync.dma_start(cd[:], cd_i[:])
        sd = cpool.tile([P, T], F32); nc.sync.dma_start(sd[:], sd_i[:])
        pm = cpool.tile([P, P], BF); nc.sync.dma_start(pm[:], pm_i[:])
        tri = cpool.tile([P, 2, T], BF)
        nc.sync.dma_start(tri[:], tri_i.rearrange("s p t -> p s t"))
        bv = cpool.tile([P, NCHUNK], F32); nc.sync.dma_start(bv[:], bv_i[:])
        ones1 = cpool.tile([P, 1], F32); nc.vector.memset(ones1[:], 1.0)
        epsb = cpool.tile([1, 1], F32); nc.vector.memset(epsb[:], EPS)
        zb = cpool.tile([P, 1], F32); nc.vector.memset(zb[:], 0.0)

        # ---- residual stream (feature-major, fp32) ----
        h = hpool.tile([P, KC, T], F32)
        nc.sync.dma_start(h[:], h0T.rearrange("(kc p) t -> p kc t", p=P))

        def rmsnorm(dst_bf):
            """dst_bf[:, kc] = h[:, kc] * rsqrt(mean_d(h^2) + EPS); norm w is
            folded into the consuming weights on the host."""
            ssq = psA.tile([1, T], F32, tag="a")
            for kc in range(KC):
                sq = tmp.tile([P, T], F32, tag="sq")
                nc.scalar.square(out=sq[:], in_=h[:, kc])
                nc.tensor.matmul(ssq[:], ones1[:], sq[:],
                                 start=(kc == 0), stop=(kc == KC - 1))
            sms = tmp.tile([1, T], F32, tag="sms")
            nc.scalar.activation(sms[:], ssq[:], mybir.ActivationFunctionType.Sqrt,
                                 bias=epsb[:], scale=1.0 / D)
            rstd = tmp.tile([1, T], F32, tag="rstd")
            nc.vector.reciprocal(rstd[:], sms[:])
            bcast = tmp.tile([P, T], F32, tag="bcast")
            nc.gpsimd.partition_broadcast(bcast[:], rstd[:])
            for kc in range(KC):
                nc.vector.tensor_mul(dst_bf[:, kc], h[:, kc], bcast[:])

        for l in range(l_use):
            # ---------------- attention norm ----------------
            hn = apool.tile([P, KC, T], BF, tag="hn")
            rmsnorm(hn)

            # ---------------- q, k, v projections ----------------
            def proj_rope(w_ap, dst):
                wt = wpool.tile([P, KC, D], BF, tag="w4")
                nc.sync.dma_start(wt[:], w_ap)
                for mc in range(KC):
                    ps = psA.tile([P, T], F32, tag="a")
                    for kc in range(KC):
                        nc.tensor.matmul(ps[:], wt[:, kc, mc * P:(mc + 1) * P],
                                         hn[:, kc], start=(kc == 0), stop=(kc == KC - 1))
                    raw = tmp.tile([P, T], BF, tag="qraw")
                    nc.scalar.copy(out=raw[:], in_=ps[:])
                    rot = psA.tile([P, T], F32, tag="a")
                    nc.tensor.matmul(rot[:], pm[:], raw[:], start=True, stop=True)
                    m1 = tmp.tile([P, T], F32, tag="m1")
                    nc.vector.tensor_mul(m1[:], raw[:], cd[:])
                    m2 = tmp.tile([P, T], F32, tag="m2")
                    nc.vector.tensor_mul(m2[:], rot[:], sd[:])
                    nc.vector.tensor_add(dst[:, mc], m1[:], m2[:])

            qT = apool.tile([P, KC, T], BF, tag="qT")
            kT = apool.tile([P, KC, T], BF, tag="kT")
            proj_rope(wk_i[l], kT)

            # K all-gather issued as soon as kT is ready (overlaps v/q proj)
            cc_kin = dram.tile([D, T], BF, tag="cckin")
            cc_kout = dram.tile([NCHUNK * D, T], BF, tag="cckout")
            nc.scalar.dma_start(cc_kin[:].rearrange("(kc p) t -> p kc t", p=P), kT[:])
            if not os.environ.get("BW_NOAG"):
                nc.gpsimd.collective_compute(
                    "AllGather", mybir.AluOpType.bypass, replica_groups=RG,
                    ins=[cc_kin.opt()], outs=[cc_kout.opt()])
            kg = kvp.tile([P, NCHUNK, KC, T], BF, tag="kg")
            for r in range(NCHUNK):
                nc.scalar.dma_start(
                    kg[:, r], cc_kout[r * D:(r + 1) * D, :].rearrange(
                        "(kc p) t -> p kc t", p=P))

            # v: token-major, strided per-head layout with a ones column at 64
            vloc = apool.tile([P, 2, H, HD + 1], BF, tag="vloc")
            wt = wpool.tile([P, KC, D], BF, tag="w4")
            nc.sync.dma_start(wt[:], wv_i[l])
            for ts in range(2):
                for nf in range(2):
                    ps = psB.tile([P, 512], F32, tag="a")
                    for kc in range(KC):
                        nc.tensor.matmul(ps[:], hn[:, kc, ts * P:(ts + 1) * P],
                                         wt[:, kc, nf * 512:(nf + 1) * 512],
                                         start=(kc == 0), stop=(kc == KC - 1))
                    nc.vector.tensor_copy(
                        vloc[:, ts, nf * 8:(nf + 1) * 8, 0:HD],
                        ps.rearrange("p (hh e) -> p hh e", e=HD))
            nc.vector.memset(vloc[:, :, :, HD:HD + 1], 1.0)

            # ---------------- V all-gather ----------------
            cc_vin = dram.tile([D, T], BF, tag="ccvin")
            cc_vout = dram.tile([NCHUNK * D, T], BF, tag="ccvout")
            ccv = cc_vin[:].flatten().rearrange(
                "(ts p hh e) -> ts p hh e", ts=2, p=P, hh=H)
            for ts in range(2):
                nc.scalar.dma_start(ccv[ts], vloc[:, ts, :, 0:HD])
            if not os.environ.get("BW_NOAG"):
                nc.gpsimd.collective_compute(
                    "AllGather", mybir.AluOpType.bypass, replica_groups=RG,
                    ins=[cc_vin.opt()], outs=[cc_vout.opt()])

            # q projection overlaps the collectives
            proj_rope(wq_i[l], qT)

            vg = kvp.tile([P, 2 * NCHUNK, H, HD + 1], BF, tag="vg")
            for r in range(NCHUNK):
                ccvo = cc_vout[r * D:(r + 1) * D, :].flatten().rearrange(
                    "(ts p hh e) -> ts p hh e", ts=2, p=P, hh=H)
                for ts in range(2):
                    nc.scalar.dma_start(vg[:, 2 * r + ts, :, 0:HD], ccvo[ts])
            nc.vector.memset(vg[:, :, :, HD:HD + 1], 1.0)

            # ---------------- attention ----------------
            attnT = apool.tile([P, KC, T], BF, tag="attnT")
            for hh in range(KC):
                # two heads (hp=0 / hp=64) share each exp: sc layout
                # [h0s0 | h0s1 | h1s0 | h1s1] along free
                avs = [psAV.tile([HD + 1, T], F32, tag="av", name=f"av{l}_{hh}_{i}") for i in range(2)]
                pairs = [("loc", None)] + [("ag", r) for r in range(NCHUNK)]
                for pi, (kind, r) in enumerate(pairs):
                    sc = psB.tile([P, 4 * T], F32, tag="a")
                    for hi in range(2):
                        hp = hi * HD
                        for sub in range(2):
                            if kind == "loc":
                                k_sl = kT[hp:hp + HD, hh, sub * P:(sub + 1) * P]
                            else:
                                k_sl = kg[hp:hp + HD, r, hh, sub * P:(sub + 1) * P]
                            nc.tensor.matmul(
                                sc[:, (2 * hi + sub) * T:(2 * hi + sub + 1) * T],
                                k_sl, qT[hp:hp + HD, hh, :], start=True, stop=True)
                    bias = zb[:] if kind == "loc" else bv[:, r:r + 1]
                    e = etmp.tile([P, 4 * T], BF, tag="e")
                    nc.scalar.activation(e[:], sc[:], mybir.ActivationFunctionType.Exp,
                                         bias=bias, scale=1.0 / np.sqrt(HD))
                    if kind == "loc":
                        ev = e.rearrange("p (hi s t) -> p hi s t", hi=2, s=2)
                        nc.vector.tensor_mul(
                            ev, ev, tri[:, None, :, :].to_broadcast([P, 2, 2, T]))
                    for hi in range(2):
                        for sub in range(2):
                            if kind == "loc":
                                v_sl = vloc[:, sub, 2 * hh + hi, :]
                            else:
                                v_sl = vg[:, 2 * r + sub, 2 * hh + hi, :]
                            nc.tensor.matmul(
                                avs[hi][:], v_sl,
                                e[:, (2 * hi + sub) * T:(2 * hi + sub + 1) * T],
                                start=(pi == 0 and sub == 0),
                                stop=(pi == len(pairs) - 1 and sub == 1))
                for hi in range(2):
                    hp = hi * HD
                    rec = tmp.tile([1, T], F32, tag="rec")
                    nc.vector.reciprocal(rec[:], avs[hi][HD:HD + 1, :])
                    brec = tmp.tile([HD, T], F32, tag="brec")
                    nc.gpsimd.partition_broadcast(brec[:], rec[:])
                    nc.vector.tensor_mul(attnT[hp:hp + HD, hh, :], avs[hi][0:HD, :], brec[:])

            # ---------------- output projection + residual ----------------
            wt = wpool.tile([P, KC, D], BF, tag="w4")
            nc.sync.dma_start(wt[:], wo_i[l])
            for dc in range(KC):
                ps = psA.tile([P, T], F32, tag="a")
                for fc in range(KC):
                    nc.tensor.matmul(ps[:], wt[:, fc, dc * P:(dc + 1) * P],
                                     attnT[:, fc], start=(fc == 0), stop=(fc == KC - 1))
                nc.vector.tensor_add(h[:, dc], ps[:], h[:, dc])

            # ---------------- FFN ----------------
            fn = apool.tile([P, KC, T], BF, tag="hn")
            rmsnorm(fn)
            yT = apool.tile([P, FC, T], BF, tag="yT")
            for mc in range(FC):
                w1t = w13p.tile([P, KC, P], BF, tag="w13")
                nc.sync.dma_start(w1t[:], w1_i[l, mc])
                g = psA.tile([P, T], F32, tag="a")
                for kc in range(KC):
                    nc.tensor.matmul(g[:], w1t[:, kc], fn[:, kc],
                                     start=(kc == 0), stop=(kc == KC - 1))
                gs = tmp.tile([P, T], BF, tag="gs")
                nc.scalar.activation(gs[:], g[:], mybir.ActivationFunctionType.Silu)
                w3t = w13p.tile([P, KC, P], BF, tag="w13")
                nc.sync.dma_start(w3t[:], w3_i[l, mc])
                u = psA.tile([P, T], F32, tag="a")
                for kc in range(KC):
                    nc.tensor.matmul(u[:], w3t[:, kc], fn[:, kc],
                                     start=(kc == 0), stop=(kc == KC - 1))
                nc.vector.tensor_mul(yT[:, mc], u[:], gs[:])
            for dc in range(KC):
                w2t = w2p.tile([P, FC, P], BF, tag="w2")
                nc.sync.dma_start(w2t[:], w2_i[l, dc])
                ps = psA.tile([P, T], F32, tag="a")
                for fc in range(FC):
                    nc.tensor.matmul(ps[:], w2t[:, fc], yT[:, fc],
                                     start=(fc == 0), stop=(fc == FC - 1))
                nc.vector.tensor_add(h[:, dc], ps[:], h[:, dc])

        # ------------- final norm + all-gather hidden + vocab-sharded logits -------
        hf = apool.tile([P, KC, T], BF, tag="hn")
        rmsnorm(hf)
        cc_hin = dram.tile([D, T], BF, tag="cchin")
        cc_hout = dram.tile([NCORES * D, T], BF, tag="cchout")
        nc.scalar.dma_start(cc_hin[:].rearrange("(kc p) t -> p kc t", p=P), hf[:])
        if not os.environ.get("BW_NOAG"):
            nc.gpsimd.collective_compute(
                "AllGather", mybir.AluOpType.bypass,
                replica_groups=[list(range(NCORES))],
                ins=[cc_hin.opt()], outs=[cc_hout.opt()])
        hfg = kvp.tile([P, NCORES, KC, T], BF, tag="hfg")
        for r in range(NCORES):
            nc.scalar.dma_start(
                hfg[:, r], cc_hout[r * D:(r + 1) * D, :].rearrange(
                    "(kc p) t -> p kc t", p=P))
        for vt in range(nvt):
            et = embp.tile([P, KC, 512], BF, tag="emb")
            nc.sync.dma_start(et[:, 0:KC // 2], emb_i[:, vt, 0:KC // 2])
            nc.sync.dma_start(et[:, KC // 2:], emb_i[:, vt, KC // 2:])
            for r in range(NCORES):
                for ts in range(2):
                    ps = psB.tile([P, 512], F32, tag="a")
                    for kc in range(KC):
                        nc.tensor.matmul(ps[:], hfg[:, r, kc, ts * P:(ts + 1) * P],
                                         et[:, kc], start=(kc == 0), stop=(kc == KC - 1))
                    ob = opool.tile([P, 512], mybir.dt.float16, tag="o")
                    if ts == 0:
                        nc.vector.tensor_copy(ob[:], ps[:])
                    else:
                        nc.scalar.copy(out=ob[:], in_=ps[:])
                    nc.sync.dma_start(
                        out_e[(2 * r + ts) * P:(2 * r + ts + 1) * P,
                              vt * 512:(vt + 1) * 512], ob[:])

    nc.compile()
    return nc


def _prep(inputs, l_use=L, v_use=V):
    """Host-side prep: fold norm weights, cast to bf16, per-core shards."""
    tokens = np.asarray(inputs["tokens"]).astype(np.int64)
    emb = np.asarray(inputs["emb"], dtype=np.float32)
    wq = np.asarray(inputs["wq"], dtype=np.float32)
    wk = np.asarray(inputs["wk"], dtype=np.float32)
    wv = np.asarray(inputs["wv"], dtype=np.float32)
    wo = np.asarray(inputs["wo"], dtype=np.float32)
    w1 = np.asarray(inputs["w1"], dtype=np.float32)
    w2 = np.asarray(inputs["w2"], dtype=np.float32)
    w3 = np.asarray(inputs["w3"], dtype=np.float32)
    anw = np.asarray(inputs["attn_norm_w"], dtype=np.float32)
    fnw = np.asarray(inputs["ffn_norm_w"], dtype=np.float32)
    finw = np.asarray(inputs["final_norm_w"], dtype=np.float32)

    def cbf(x):
        return np.ascontiguousarray(x.astype(BF16))

    def wlayout(w):
        # [L, D, F] -> [L, P, KC, F]: contiguous per-partition rows
        return np.ascontiguousarray(
            w.reshape(l_use, KC, P, -1).transpose(0, 2, 1, 3))

    wq_f = wlayout(cbf(wq[:l_use] * anw[:l_use, :, None]))
    wk_f = wlayout(cbf(wk[:l_use] * anw[:l_use, :, None]))
    wv_f = wlayout(cbf(wv[:l_use] * anw[:l_use, :, None]))
    wo_f = wlayout(cbf(wo[:l_use]))
    w1_f = (w1[:l_use] * fnw[:l_use, :, None]).astype(BF16)
    w3_f = (w3[:l_use] * fnw[:l_use, :, None]).astype(BF16)
    w1c = np.ascontiguousarray(
        w1_f.reshape(l_use, KC, P, FC, P).transpose(0, 3, 2, 1, 4))
    w3c = np.ascontiguousarray(
        w3_f.reshape(l_use, KC, P, FC, P).transpose(0, 3, 2, 1, 4))
    w2c = np.ascontiguousarray(
        w2[:l_use].astype(BF16).reshape(l_use, FC, P, KC, P).transpose(0, 3, 2, 1, 4))
    v_use = int(os.environ.get("BW_VOCAB", V))
    nvt = max(1, (v_use + NCORES * 512 - 1) // (NCORES * 512))
    vsh = nvt * 512
    embf = (emb * finw[None, :]).astype(BF16).T  # [D, V]
    if NCORES * vsh > embf.shape[1]:
        embf = np.pad(embf, ((0, 0), (0, NCORES * vsh - embf.shape[1])))
    embf = embf[:, :NCORES * vsh]
    embT_shards = [
        np.ascontiguousarray(
            embf[:, c * vsh:(c + 1) * vsh].reshape(KC, P, nvt, 512).transpose(1, 2, 0, 3))
        for c in range(NCORES)]

    permf = np.zeros((P, P), np.float32)
    for i in range(P // 2):
        permf[2 * i + 1, 2 * i] = -1.0
        permf[2 * i, 2 * i + 1] = 1.0
    permb = permf.astype(BF16)
    tri = np.zeros((2, P, T), np.float32)
    for sub in range(2):
        for p in range(P):
            tri[sub, p, :] = (sub * P + p) <= np.arange(T)
    trib = tri.astype(BF16)

    inv = 1.0 / (THETA ** (np.arange(0, HD, 2, dtype=np.float32) / HD))  # [32]

    in_maps = []
    for core in range(NCORES):
        b, c = core // NCHUNK, core % NCHUNK
        toks = tokens[b, c * T:(c + 1) * T]
        h0T = np.ascontiguousarray(emb[toks].T)  # [D, T] fp32
        pos = (c * T + np.arange(T)).astype(np.float32)
        ang = pos[None, :] * inv[:, None]        # [32, T]
        cdup = np.empty((P, T), np.float32)
        sdup = np.empty((P, T), np.float32)
        for p in range(P):
            f = (p % HD) // 2
            cdup[p] = np.cos(ang[f])
            sdup[p] = np.sin(ang[f])
        biasv = np.zeros((P, NCHUNK), np.float32)
        for j in range(NCHUNK):
            if j >= c:
                biasv[:, j] = NEG
        in_maps.append({
            "h0T": h0T, "wq": wq_f, "wk": wk_f, "wv": wv_f, "wo": wo_f,
            "w1c": w1c, "w3c": w3c, "w2c": w2c, "embT": embT_shards[core],
            "cdup": cdup, "sdup": sdup, "perm": permb, "tri": trib,
            "biasv": biasv,
        })
    return in_maps


def _get_nc(l_use=L, v_use=V):
    key = (l_use, v_use)
    if key not in _CACHE:
        _CACHE[key] = _build(l_use, v_use)
    return _CACHE[key]


def run_device(in_maps, l_use=L, v_use=V):
    nc = _get_nc(l_use, v_use)
    res = bass_utils.run_bass_kernel_spmd(
        nc, in_maps, core_ids=list(range(NCORES)))
    return res


def kernel(**inputs) -> np.ndarray:
    l_use = int(os.environ.get("BW_LAYERS", L))
    v_use = int(os.environ.get("BW_VOCAB", V))
    in_maps = _prep(inputs, l_use)
    res = run_device(in_maps, l_use, v_use)
    nvt = max(1, (v_use + NCORES * 512 - 1) // (NCORES * 512))
    vsh = nvt * 512
    full = np.empty((B * S, NCORES * vsh), np.float32)
    for core in range(NCORES):
        full[:, core * vsh:(core + 1) * vsh] = res.results[core]["logits_loc"]
    return full[:, :v_use].reshape(B, S, v_use)


# ---------------------------------------------------------------------------
# Timing helpers (used by test.py; the grading harness only calls kernel()).
# ---------------------------------------------------------------------------

def make_runner(in_maps, l_use=L, v_use=V, chain=1):
    """Return (run, out_names, out_avals). run() dispatches one NEFF execution
    on all 8 cores with device-resident inputs and returns per-core outputs."""
    import jax
    import jax.numpy as jnp
    from jax.sharding import Mesh, PartitionSpec
    from jax.experimental.shard_map import shard_map
    from concourse.bass2jax import (_bass_exec_p, install_neuronx_cc_hook,
                                    partition_id_tensor)
    import concourse.mybir as mb

    nc = _get_nc(l_use, v_use)
    install_neuronx_cc_hook()
    partition_name = nc.partition_id_tensor.name if nc.partition_id_tensor else None
    in_names, out_names, out_avals = [], [], []
    for alloc in nc.m.functions[0].allocations:
        if not isinstance(alloc, mb.MemoryLocationSet):
            continue
        name = alloc.memorylocations[0].name
        if alloc.kind == "ExternalInput":
            if name != partition_name:
                in_names.append(name)
        elif alloc.kind == "ExternalOutput":
            out_names.append(name)
            out_avals.append(jax.core.ShapedArray(
                tuple(alloc.tensor_shape), mb.dt.np(alloc.dtype)))
    n_params = len(in_names)
    all_names = tuple(in_names + out_names +
                      ([partition_name] if partition_name else []))

    def _once(args, zeros):
        operands = list(args) + list(zeros)
        if partition_name is not None:
            operands.append(partition_id_tensor())
        return tuple(_bass_exec_p.bind(
            *operands, out_avals=tuple(out_avals), in_names=all_names,
            out_names=tuple(out_names), lowering_input_output_aliases=(),
            sim_require_finite=True, sim_require_nnan=True, nc=nc))

    def _body(*flat):
        args, outs = flat[:n_params], flat[n_params:]
        for _ in range(chain):
            outs = _once(args, outs)
        return outs

    from jax.sharding import NamedSharding
    devices = jax.devices()[:NCORES]
    mesh = Mesh(np.asarray(devices), ("core",))
    n_outs = len(out_names)
    in_specs = (PartitionSpec("core"),) * (n_params + n_outs)
    out_specs = (PartitionSpec("core"),) * n_outs
    fn = jax.jit(shard_map(_body, mesh=mesh, in_specs=in_specs,
                           out_specs=out_specs, check_rep=False),
                 keep_unused=True)

    def shard(a):
        sh = NamedSharding(mesh, PartitionSpec("core", *([None] * (a.ndim - 1))))
        return jax.device_put(a, sh)

    concat_in = [shard(np.concatenate(
        [np.asarray(in_maps[c][nm]) for c in range(NCORES)], axis=0))
        for nm in in_names]
    zeros = [shard(np.zeros((NCORES * a.shape[0], *a.shape[1:]), a.dtype))
             for a in out_avals]

    def run():
        return fn(*concat_in, *zeros)

    return run, out_names, out_avals

